# revision 1
# baseline (speedup 1.0000x reference)
"""Trainium2 Bass kernel for a 2-layer Longformer-style encoder.

Model: B=2, S=2048, F=438, H=768, NH=12, HD=64, one-sided window w=32, L=2.

Sharding: 8 cores, data-parallel over (batch, sequence-quarter). Each core
computes 512 output tokens from a 640-token local window (64-token halo on
each side covers the 2-layer receptive field), so no collectives are needed.

Device algorithm per core (uniform SPMD, 640 local tokens):
  - x0 = srcT_pad.T @ W_embT + (pos_emb + b_emb)           [token-major f32]
  - per layer:
      xT   = transpose(x)  bf16                             [feature-major]
      qT   = W_qT'.T @ xT (+bq'), scaled by HD^-0.5 on host [feature-major]
      kTp  = W_kT.T @ xT (+bk), written at free-offset 32 into a
             704-wide padded buffer                         [feature-major]
      V_sh = shifted-window GEMM: tile i holds local tokens
             [128i-32, 128i+96)                             [token-major]
      per (query tile t, head h):
        ST[n,q] = kTp[:, span_t].T-style matmul             [n-major scores]
        ST += mask_t (host-baked band+boundary, n-major)
        P = exp(ST)        (no max-subtraction; scores are small)
        den[h,q] += onehot-ones matmuls over P
        ctxT[64h:64h+64, tile] += V_sh.T @ P                [feature-major]
      fc: F = ctxT.T @ W_fcT; per-head-block F = F*recip + residual
      LN1 -> x1 (token-major f32), transpose -> x1T bf16
      H1T = relu(W_1T.T @ x1T + b1)                         [feature-major]
      F2 = H1T.T @ W_2T (+b2) + x1; LN2 -> x2
  - out = x2[64:576]
"""

import numpy as np
import ml_dtypes

B, S, F_DIM, H, NH, HD, W_ONE, L = 2, 2048, 438, 768, 12, 64, 32, 2
NCORES = 8
CHUNK = 512          # output tokens per core
HALO = 64            # per side
T_LOC = CHUNK + 2 * HALO   # 640 local tokens
NT = T_LOC // 128          # 5 query tiles
KPAD = T_LOC + 64          # 704 padded key width
SPAN = 192                 # keys per query tile (128 + 2*32)
FK = 512                   # padded embedding contraction (438 -> 512)
# Large enough that masked positions contribute ~e^-50 ~ 2e-22 (negligible),
# small enough that a fully-masked (pad) query row keeps a nonzero softmax
# denominator -> no inf/NaN anywhere (pad rows are discarded on output).
MASK_NEG = -50.0

bf16 = ml_dtypes.bfloat16


def _np(x):
    return np.asarray(x)


def host_prep(inputs):
    """Split full inputs into shared weight arrays + per-core arrays."""
    src_seq = _np(inputs["src_seq"]).astype(np.float32)
    src_pos = _np(inputs["src_pos"]).astype(np.int32)
    pos_table = _np(inputs["pos_table"]).astype(np.float32)

    shared = {}
    qscale = float(HD) ** -0.5

    W_emb = _np(inputs["W_emb"]).astype(np.float32)        # [H, F]
    WembT = np.zeros((FK, H), np.float32)
    WembT[:F_DIM] = W_emb.T
    shared["wembT"] = WembT.astype(bf16)

    for l in range(L):
        Wq = _np(inputs["Wq"])[l].astype(np.float32)
        Wk = _np(inputs["Wk"])[l].astype(np.float32)
        Wv = _np(inputs["Wv"])[l].astype(np.float32)
        Wfc = _np(inputs["Wfc"])[l].astype(np.float32)
        W1 = _np(inputs["W1"])[l].astype(np.float32)
        W2 = _np(inputs["W2"])[l].astype(np.float32)
        shared[f"wqT{l}"] = (Wq.T * qscale).astype(bf16)   # [H_in, H_out]
        shared[f"wkT{l}"] = Wk.T.astype(bf16)
        shared[f"wvT{l}"] = Wv.T.astype(bf16)
        shared[f"wfcT{l}"] = Wfc.T.astype(bf16)
        shared[f"w1T{l}"] = W1.T.astype(bf16)
        shared[f"w2T{l}"] = W2.T.astype(bf16)
        shared[f"bq{l}"] = (_np(inputs["bq"])[l].astype(np.float32) * qscale)
        shared[f"bk{l}"] = _np(inputs["bk"])[l].astype(np.float32)
        shared[f"bv{l}"] = _np(inputs["bv"])[l].astype(np.float32)
        shared[f"bfc{l}"] = _np(inputs["bfc"])[l].astype(np.float32)
        shared[f"b1{l}"] = _np(inputs["b1"])[l].astype(np.float32)
        shared[f"b2{l}"] = _np(inputs["b2"])[l].astype(np.float32)
        shared[f"ln1g{l}"] = _np(inputs["ln1_g"])[l].astype(np.float32)
        shared[f"ln1b{l}"] = _np(inputs["ln1_b"])[l].astype(np.float32)
        shared[f"ln2g{l}"] = _np(inputs["ln2_g"])[l].astype(np.float32)
        shared[f"ln2b{l}"] = _np(inputs["ln2_b"])[l].astype(np.float32)

    b_emb = _np(inputs["b_emb"]).astype(np.float32)

    per_core = []
    for c in range(NCORES):
        b, q = divmod(c, NCORES // B)
        gstart = q * CHUNK - HALO
        lo, hi = max(gstart, 0), min(gstart + T_LOC, S)

        src_halo = np.zeros((T_LOC, F_DIM), np.float32)
        src_halo[lo - gstart: hi - gstart] = src_seq[b, lo:hi]
        srcT = np.zeros((FK, T_LOC), np.float32)
        srcT[:F_DIM] = src_halo.T

        pos_emb = np.zeros((T_LOC, H), np.float32)
        pos_emb[lo - gstart: hi - gstart] = pos_table[src_pos[b, lo:hi]]
        pos_emb += b_emb[None, :]

        # n-major masks per query tile: mask[n, q] for span keys
        # local key = 128t - 32 + n, query local = 128t + q.
        maskA = np.full((128, NT, 128), MASK_NEG, np.float32)
        maskB = np.full((64, NT, 128), MASK_NEG, np.float32)
        for t in range(NT):
            n = np.arange(SPAN)[:, None]
            qq = np.arange(128)[None, :]
            kl = 128 * t - 32 + n
            kg = gstart + kl
            band = np.abs(kl - (128 * t + qq)) <= W_ONE
            valid = band & (kl >= 0) & (kl < T_LOC) & (kg >= 0) & (kg < S)
            m = np.where(valid, 0.0, MASK_NEG).astype(np.float32)
            maskA[:, t, :] = m[:128]
            maskB[:, t, :] = m[128:]

        per_core.append({
            "srcT": srcT.astype(bf16),
            "pos_emb": pos_emb,
            "maskA": maskA,
            "maskB": maskB,
        })

    # constants
    shared["ident"] = np.eye(128, dtype=np.float32)

    flags = {}
    for l in range(L):
        for nm in ("bq", "bk", "bv", "bfc", "b1", "b2"):
            flags[f"{nm}{l}"] = not np.allclose(shared[f"{nm}{l}"], 0.0)
        for nm in ("ln1", "ln2"):
            flags[f"{nm}{l}"] = not (
                np.allclose(shared[f"{nm}g{l}"], 1.0)
                and np.allclose(shared[f"{nm}b{l}"], 0.0)
            )
    return shared, per_core, flags


def assemble(core_outs):
    out = np.zeros((B, S, H), np.float32)
    for c in range(NCORES):
        b, q = divmod(c, NCORES // B)
        out[b, q * CHUNK:(q + 1) * CHUNK] = core_outs[c]
    return out


# ---------------------------------------------------------------------------
# Bass program
# ---------------------------------------------------------------------------

def _legalize_waits(nc):
    """This container's walrus codegen accepts only ONE sync-wait per compute
    instruction ("Too many sync wait commands"). Tile's scheduler emits
    multi-wait instructions, so split: keep the last wait on the instruction
    and carry earlier ones on same-engine NoOps inserted right before it."""
    import concourse.mybir as mybir

    for fn in nc.m.functions:
        for blk in fn.blocks:
            out = []
            changed = False
            for inst in blk.instructions:
                si = getattr(inst, "sync_info", None)
                waits = list(si.on_wait) if si is not None and si.on_wait else []
                if len(waits) > 1 and not isinstance(
                        inst, mybir.InstEventSemaphore):
                    for j, w in enumerate(waits[:-1]):
                        # NoOp lowers through the v3 codegen only; Activation
                        # and Pool go through v2 (no InstISA nop) -> use a
                        # 1-wait Drain there instead.
                        if inst.engine in (mybir.EngineType.Activation,
                                           mybir.EngineType.Pool):
                            nop = mybir.InstDrain(
                                name=f"{inst.name}-w{j}", ins=[], outs=[])
                        else:
                            nop = mybir.InstNoOp(
                                name=f"{inst.name}-w{j}", ins=[], outs=[])
                        nop.engine = inst.engine
                        nop.sync_info = mybir.SyncInfo(on_wait=[w], on_update=[])
                        out.append(nop)
                    inst.sync_info = mybir.SyncInfo(
                        on_wait=[waits[-1]], on_update=list(si.on_update or []))
                    changed = True
                out.append(inst)
            if changed:
                blk.instructions = out


def _act_reciprocal(nc, mybir, out, in_):
    """ACT-engine LUT reciprocal. bass raises on ActivationFunctionType.
    Reciprocal citing accuracy issues, but measured on this hardware it is
    ~1e-5 relative over [1e-6, 1e3] - plenty for softmax denominators."""
    eng = nc.scalar
    inputs = [eng.lower_ap(in_)]
    for arg in (0.0, 1.0, 0.0):
        inputs.append(mybir.ImmediateValue(dtype=mybir.dt.float32, value=arg))
    return eng.add_instruction(mybir.InstActivation(
        name=nc.get_next_instruction_name(),
        func=mybir.ActivationFunctionType.Reciprocal,
        ins=inputs, outs=[eng.lower_ap(out)]))


def build_program(flags):
    import concourse.bass as bass
    import concourse.mybir as mybir
    import concourse.tile as tile

    f32 = mybir.dt.float32
    bf = mybir.dt.bfloat16
    AF = mybir.ActivationFunctionType
    ALU = mybir.AluOpType

    nc = bass.Bass()
    FT = H // 128          # 6 feature tiles
    KTE = FK // 128        # 4 embedding contraction tiles

    # ---- DRAM tensors ----
    D = {}
    names = []

    def din(name, shape, dt):
        D[name] = nc.dram_tensor(name, shape, dt, kind="ExternalInput")
        names.append(name)

    din("srcT", [FK, T_LOC], bf)
    din("pos_emb", [T_LOC, H], f32)
    din("maskA", [128, NT, 128], f32)
    din("maskB", [64, NT, 128], f32)
    din("ident", [128, 128], f32)
    din("wembT", [FK, H], bf)
    for l in range(L):
        for nm in ("wqT", "wkT", "wvT", "wfcT", "w1T", "w2T"):
            din(f"{nm}{l}", [H, H], bf)
        for nm in ("bq", "bk", "bv", "bfc", "b1", "b2"):
            if flags[f"{nm}{l}"]:
                din(f"{nm}{l}", [H], f32)
        for nm in ("ln1", "ln2"):
            if flags[f"{nm}{l}"]:
                din(f"{nm}g{l}", [H], f32)
                din(f"{nm}b{l}", [H], f32)
    out_d = nc.dram_tensor("out", [CHUNK, H], f32, kind="ExternalOutput")

    def bcast_ap(dram, n):
        return bass.AP(tensor=dram.tensor, offset=dram.offset, ap=[[0, 128], [1, n]])

    with tile.TileContext(nc) as tc:
        import contextlib
        with contextlib.ExitStack() as ctx:
            consts = ctx.enter_context(tc.tile_pool(name="consts", bufs=1))
            acts = ctx.enter_context(tc.tile_pool(name="acts", bufs=1))
            work = ctx.enter_context(tc.tile_pool(name="work", bufs=2))
            ppool = ctx.enter_context(tc.tile_pool(name="pp", bufs=4))
            rpool = ctx.enter_context(tc.tile_pool(name="rp", bufs=2))
            spool = ctx.enter_context(tc.tile_pool(name="sp", bufs=4))
            psg = ctx.enter_context(tc.tile_pool(name="psg", bufs=2, space="PSUM"))
            pst = ctx.enter_context(tc.tile_pool(name="pst", bufs=2, space="PSUM"))
            psc = ctx.enter_context(tc.tile_pool(name="psc", bufs=2, space="PSUM"))
            pstr = ctx.enter_context(tc.tile_pool(name="pstr", bufs=2, space="PSUM"))

            # ---- constants / inputs to SBUF ----
            ident_sb = consts.tile([128, 128], f32)
            nc.sync.dma_start(out=ident_sb, in_=D["ident"][:, :])
            ones64 = consts.tile([128, 64], bf)
            nc.vector.memset(ones64, 1.0)
            eps_sb = consts.tile([128, 1], f32)
            nc.vector.memset(eps_sb, 1e-5)

            maskA_sb = consts.tile([128, NT, 128], f32)
            nc.sync.dma_start(out=maskA_sb, in_=D["maskA"][:, :, :])
            maskB_sb = consts.tile([64, NT, 128], f32)
            nc.sync.dma_start(out=maskB_sb, in_=D["maskB"][:, :, :])

            srcT_sb = consts.tile([128, KTE, T_LOC], bf)
            for kt in range(KTE):
                nc.sync.dma_start(out=srcT_sb[:, kt, :],
                                  in_=D["srcT"][kt * 128:(kt + 1) * 128, :])
            pos_sb = consts.tile([128, NT, H], f32)
            for t in range(NT):
                nc.sync.dma_start(out=pos_sb[:, t, :],
                                  in_=D["pos_emb"][t * 128:(t + 1) * 128, :])

            # weights stream through a rotating pool: each is used once
            wpool = ctx.enter_context(tc.tile_pool(name="wpool", bufs=3))

            def load_w(name, kt=FT):
                wt = wpool.tile([128, kt, H], bf, name=f"{name}_sb", tag="wt")
                for k in range(kt):
                    nc.sync.dma_start(out=wt[:, k, :],
                                      in_=D[name][k * 128:(k + 1) * 128, :])
                return wt

            BIAS = {}
            for l in range(L):
                for nm in ("bq", "bk", "b1"):  # per-partition, feature-major
                    if flags[f"{nm}{l}"]:
                        BIAS[f"{nm}{l}"] = consts.tile([128, FT], f32, name=f"{nm}{l}_sb")
                        nc.sync.dma_start(
                            out=BIAS[f"{nm}{l}"],
                            in_=D[f"{nm}{l}"].rearrange("(kt p) -> p kt", p=128))
                for nm in ("bv", "bfc", "b2"):  # broadcast, token-major
                    if flags[f"{nm}{l}"]:
                        BIAS[f"{nm}{l}"] = consts.tile([128, H], f32, name=f"{nm}{l}_sb")
                        nc.sync.dma_start(
                            out=BIAS[f"{nm}{l}"], in_=bcast_ap(D[f"{nm}{l}"], H))
                for nm in ("ln1", "ln2"):
                    if flags[f"{nm}{l}"]:
                        for gb in ("g", "b"):
                            BIAS[f"{nm}{gb}{l}"] = consts.tile([128, H], f32, name=f"{nm}{gb}{l}_sb")
                            nc.sync.dma_start(
                                out=BIAS[f"{nm}{gb}{l}"],
                                in_=bcast_ap(D[f"{nm}{gb}{l}"], H))

            # ---- persistent activations ----
            x_tok = acts.tile([128, NT, H], f32)          # token-major f32
            qT = acts.tile([128, FT, T_LOC], bf)
            kTp = acts.tile([128, FT, KPAD], bf)
            V_sh = acts.tile([128, FT, H], bf)            # 6 shifted token tiles
            ctxT = acts.tile([128, FT, T_LOC], bf)
            H1T = acts.tile([128, FT, T_LOC], bf)

            # xT: feature-major bf16 with 32-col zero pad on each side (cols
            # [32, 672) hold tokens [0, 640)); a fresh generation per
            # transpose-set so the pool tracks lifetimes.
            def new_xT(name):
                t_ = acts.tile([128, FT, H], bf, name=name, tag="xTslot")
                nc.vector.memset(t_[:, :, 0:32], 0.0)
                nc.vector.memset(t_[:, :, 32 + T_LOC:H], 0.0)
                return t_

            # ---- embedding ----
            wembT_sb = load_w("wembT", kt=KTE)
            for t in range(NT):
                for c0 in (0, 384):
                    ps = psg.tile([128, 384], f32, tag="gemm")
                    for kt in range(KTE):
                        nc.tensor.matmul(
                            ps, srcT_sb[:, kt, t * 128:(t + 1) * 128],
                            wembT_sb[:, kt, c0:c0 + 384],
                            start=(kt == 0), stop=(kt == KTE - 1))
                    nc.vector.tensor_add(
                        x_tok[:, t, c0:c0 + 384], ps, pos_sb[:, t, c0:c0 + 384])

            def transpose_set(dst, t, gscale=None, gbias=None):
                """PE-transpose x_tok tile t into dst[:, :, 128t:+128] (bf16)."""
                for g in range(2):
                    n_g = 4 if g == 0 else 2
                    trp = pstr.tile([128, 512], f32, tag="tr")
                    for j in range(n_g):
                        ft = 4 * g + j
                        nc.tensor.transpose(
                            trp[:, j * 128:(j + 1) * 128],
                            x_tok[:, t, ft * 128:(ft + 1) * 128], ident_sb)
                    src = trp[:, 0:n_g * 128].rearrange("p (a b) -> p a b", b=128)
                    nc.vector.tensor_copy(
                        dst[:, 4 * g:4 * g + n_g, 32 + t * 128:32 + (t + 1) * 128],
                        src)

            xT = new_xT("x0T")
            for t in range(NT):
                transpose_set(xT, t)

            # ---- layers ----
            for l in range(L):
                # kTp pad memsets
                nc.vector.memset(kTp[:, :, 0:32], 0.0)
                nc.vector.memset(kTp[:, :, 32 + T_LOC:KPAD], 0.0)

                # q/k GEMMs (feature-major outputs)
                wq = load_w(f"wqT{l}")
                wk = load_w(f"wkT{l}")
                for ft in range(FT):
                    for c0 in (0, 320):
                        psq = psg.tile([128, 384], f32, tag="gemm")
                        for kt in range(FT):
                            nc.tensor.matmul(
                                psq[:, 0:320], wq[:, kt, ft * 128:(ft + 1) * 128],
                                xT[:, kt, 32 + c0:32 + c0 + 320],
                                start=(kt == 0), stop=(kt == FT - 1))
                        if flags[f"bq{l}"]:
                            nc.vector.tensor_scalar_add(
                                qT[:, ft, c0:c0 + 320], psq[:, 0:320],
                                BIAS[f"bq{l}"][:, ft:ft + 1])
                        else:
                            nc.vector.tensor_copy(
                                qT[:, ft, c0:c0 + 320], psq[:, 0:320])
                        psk = psg.tile([128, 384], f32, tag="gemm")
                        for kt in range(FT):
                            nc.tensor.matmul(
                                psk[:, 0:320], wk[:, kt, ft * 128:(ft + 1) * 128],
                                xT[:, kt, 32 + c0:32 + c0 + 320],
                                start=(kt == 0), stop=(kt == FT - 1))
                        if flags[f"bk{l}"]:
                            nc.vector.tensor_scalar_add(
                                kTp[:, ft, 32 + c0:32 + c0 + 320], psk[:, 0:320],
                                BIAS[f"bk{l}"][:, ft:ft + 1])
                        else:
                            nc.vector.tensor_copy(
                                kTp[:, ft, 32 + c0:32 + c0 + 320], psk[:, 0:320])

                # V GEMM: shifted token windows (xT is padded, so window i is
                # simply padded cols [128i, 128i+128) -> uniform M=128)
                wv = load_w(f"wvT{l}")
                for i in range(FT):
                    for c0 in (0, 384):
                        psv = psg.tile([128, 384], f32, tag="gemm")
                        for kt in range(FT):
                            nc.tensor.matmul(
                                psv, xT[:, kt, 128 * i:128 * i + 128],
                                wv[:, kt, c0:c0 + 384],
                                start=(kt == 0), stop=(kt == FT - 1))
                        if flags[f"bv{l}"]:
                            nc.vector.tensor_add(
                                V_sh[:, i, c0:c0 + 384], psv,
                                BIAS[f"bv{l}"][:, c0:c0 + 384])
                        else:
                            nc.vector.tensor_copy(V_sh[:, i, c0:c0 + 384], psv)

                # attention
                for t in range(NT):
                    for hp in range(FT):
                        cps = psc.tile([128, 256], f32, tag="ctx")
                        for hs in range(2):
                            h = 2 * hp + hs
                            po = 64 * hs
                            stp = pst.tile([128, 256], f32, tag="st")
                            nc.tensor.matmul(
                                stp[:, 0:128],
                                kTp[po:po + 64, hp, 128 * t:128 * t + 128],
                                qT[po:po + 64, hp, 128 * t:128 * t + 128],
                                start=True, stop=True)
                            nc.tensor.matmul(
                                stp[0:64, 128:256],
                                kTp[po:po + 64, hp, 128 * t + 128:128 * t + 192],
                                qT[po:po + 64, hp, 128 * t:128 * t + 128],
                                start=True, stop=True)
                            nc.vector.tensor_add(
                                stp[:, 0:128], stp[:, 0:128], maskA_sb[:, t, :])
                            nc.vector.tensor_add(
                                stp[0:64, 128:256], stp[0:64, 128:256],
                                maskB_sb[:, t, :])
                            p_hi = ppool.tile([128, 128], bf, tag="p_hi")
                            p_lo = ppool.tile([64, 128], bf, tag="p_lo")
                            nc.scalar.activation(p_hi, stp[:, 0:128], AF.Exp)
                            nc.scalar.activation(p_lo, stp[0:64, 128:256], AF.Exp)
                            # denominator, replicated over the head's 64 rows
                            nc.tensor.matmul(
                                cps[po:po + 64, 128:256], ones64[0:128, :], p_hi,
                                start=True, stop=False)
                            nc.tensor.matmul(
                                cps[po:po + 64, 128:256], ones64[0:64, :], p_lo,
                                start=False, stop=True)
                            # ctx (unnormalized)
                            nc.tensor.matmul(
                                cps[po:po + 64, 0:128],
                                V_sh[0:128, t, 64 * h:64 * h + 64], p_hi,
                                start=True, stop=False)
                            nc.tensor.matmul(
                                cps[po:po + 64, 0:128],
                                V_sh[0:64, t + 1, 64 * h:64 * h + 64], p_lo,
                                start=False, stop=True)
                        rb = rpool.tile([128, 128], f32, tag="rb")
                        # clamp pad-row denominators (~e-20) into LUT range
                        nc.vector.tensor_scalar_max(
                            cps[:, 128:256], cps[:, 128:256], 1e-6)
                        _act_reciprocal(nc, mybir, rb, cps[:, 128:256])
                        nc.vector.tensor_tensor(
                            out=ctxT[:, hp, 128 * t:128 * t + 128],
                            in0=cps[:, 0:128], in1=rb, op=ALU.mult)

                # fc + residual + LN1 (+ transposes into a fresh xT gen)
                wfc = load_w(f"wfcT{l}")
                xT = new_xT(f"x1T{l}")
                for t in range(NT):
                    F = work.tile([128, H], f32, tag="F")
                    for c0 in (0, 384):
                        ps = psg.tile([128, 384], f32, tag="gemm")
                        for kt in range(FT):
                            nc.tensor.matmul(
                                ps, ctxT[:, kt, 128 * t:128 * t + 128],
                                wfc[:, kt, c0:c0 + 384],
                                start=(kt == 0), stop=(kt == FT - 1))
                        nc.vector.tensor_add(
                            F[:, c0:c0 + 384], ps, x_tok[:, t, c0:c0 + 384])
                        if flags[f"bfc{l}"]:
                            nc.vector.tensor_add(
                                F[:, c0:c0 + 384], F[:, c0:c0 + 384],
                                BIAS[f"bfc{l}"][:, c0:c0 + 384])
                    _layernorm(nc, tc, spool, F, x_tok, t, eps_sb,
                               BIAS.get(f"ln1g{l}"), BIAS.get(f"ln1b{l}"),
                               f32, AF, ALU)
                    transpose_set(xT, t)  # x1T reuses the xT slot

                # FFN
                w1 = load_w(f"w1T{l}")
                for ft in range(FT):
                    for c0 in (0, 320):
                        ps = psg.tile([128, 384], f32, tag="gemm")
                        for kt in range(FT):
                            nc.tensor.matmul(
                                ps[:, 0:320], w1[:, kt, ft * 128:(ft + 1) * 128],
                                xT[:, kt, 32 + c0:32 + c0 + 320],
                                start=(kt == 0), stop=(kt == FT - 1))
                        bias = (BIAS[f"b1{l}"][:, ft:ft + 1]
                                if flags[f"b1{l}"] else 0.0)
                        nc.scalar.activation(
                            H1T[:, ft, c0:c0 + 320], ps[:, 0:320], AF.Relu,
                            bias=bias)
                w2 = load_w(f"w2T{l}")
                if l < L - 1:
                    xT = new_xT(f"x2T{l}")
                for t in range(NT):
                    F2 = work.tile([128, H], f32, tag="F")
                    for c0 in (0, 384):
                        ps = psg.tile([128, 384], f32, tag="gemm")
                        for kt in range(FT):
                            nc.tensor.matmul(
                                ps, H1T[:, kt, 128 * t:128 * t + 128],
                                w2[:, kt, c0:c0 + 384],
                                start=(kt == 0), stop=(kt == FT - 1))
                        nc.vector.tensor_add(
                            F2[:, c0:c0 + 384], ps, x_tok[:, t, c0:c0 + 384])
                        if flags[f"b2{l}"]:
                            nc.vector.tensor_add(
                                F2[:, c0:c0 + 384], F2[:, c0:c0 + 384],
                                BIAS[f"b2{l}"][:, c0:c0 + 384])
                    _layernorm(nc, tc, spool, F2, x_tok, t, eps_sb,
                               BIAS.get(f"ln2g{l}"), BIAS.get(f"ln2b{l}"),
                               f32, AF, ALU)
                    if l < L - 1:
                        transpose_set(xT, t)
                    else:
                        lo = max(128 * t, HALO) - 128 * t
                        hi = min(128 * t + 128, HALO + CHUNK) - 128 * t
                        nc.sync.dma_start(
                            out=out_d[128 * t + lo - HALO:128 * t + hi - HALO, :],
                            in_=x_tok[lo:hi, t, :])

    _legalize_waits(nc)
    return nc, names


def _layernorm(nc, tc, spool, F, x_tok, t, eps_sb, g_bc, b_bc, f32, AF, ALU):
    stats = spool.tile([128, 3, 6], f32, tag="stats")
    for sg in range(3):
        nc.vector.bn_stats(stats[:, sg, :], F[:, sg * 256:(sg + 1) * 256])
    mv = spool.tile([128, 2], f32, tag="mv")
    nc.vector.bn_aggr(mv, stats)
    sd = spool.tile([128, 1], f32, tag="sd")
    nc.scalar.activation(sd, mv[:, 1:2], AF.Sqrt, bias=eps_sb[:, 0:1])
    rstd = spool.tile([128, 1], f32, tag="rstd")
    nc.vector.reciprocal(rstd, sd)
    nc.vector.tensor_scalar(
        out=x_tok[:, t, :], in0=F, scalar1=mv[:, 0:1], scalar2=rstd,
        op0=ALU.subtract, op1=ALU.mult)
    if g_bc is not None:
        nc.vector.tensor_tensor(
            out=x_tok[:, t, :], in0=x_tok[:, t, :], in1=g_bc, op=ALU.mult)
        nc.vector.tensor_tensor(
            out=x_tok[:, t, :], in0=x_tok[:, t, :], in1=b_bc, op=ALU.add)


def run_on_device(shared, per_core, flags, trace=False):
    from concourse.bass_utils import run_bass_kernel_spmd

    nc, names = build_program(flags)
    in_maps = []
    for c in range(NCORES):
        m = {}
        for n in names:
            src = per_core[c] if n in per_core[c] else shared
            m[n] = np.ascontiguousarray(src[n])
        in_maps.append(m)
    res = run_bass_kernel_spmd(nc, in_maps, core_ids=list(range(NCORES)),
                               trace=trace)
    return [r["out"] for r in res.results], res


def kernel(**inputs):
    shared, per_core, flags = host_prep(inputs)
    core_outs, _ = run_on_device(shared, per_core, flags)
    return assemble(core_outs)




# revision 21
# speedup vs baseline: 1.2797x; 1.2797x over previous
"""Trainium2 Bass kernel for a 2-layer Longformer-style encoder.

Model: B=2, S=2048, F=438, H=768, NH=12, HD=64, one-sided window w=32, L=2.

Sharding: 8 cores, data-parallel over (batch, sequence-quarter). Each core
computes 512 output tokens from a 640-token local window (64-token halo on
each side covers the 2-layer receptive field), so no collectives are needed.

Device algorithm per core (uniform SPMD, 640 local tokens):
  - x0 = srcT_pad.T @ W_embT + (pos_emb + b_emb)           [token-major f32]
  - per layer:
      xT   = transpose(x)  bf16                             [feature-major]
      qT   = W_qT'.T @ xT (+bq'), scaled by HD^-0.5 on host [feature-major]
      kTp  = W_kT.T @ xT (+bk), written at free-offset 32 into a
             704-wide padded buffer                         [feature-major]
      V_aug = shifted-window GEMM into per-head 128-col slots
             [V_h | ones]; window i holds local tokens
             [128i-32, 128i+96)                             [token-major]
      per (query tile t, group g of 4 heads):
        stp[128,1024] psum <- band mask via bf16 ident matmul (start=True)
        per head slot: hi/lo QK^T matmuls accumulate on top  [n-major]
        P = exp(stp)  one ACT op, bf16   (scores are small; no max-sub)
        per head slot: cps[:,j] = V_aug.T @ P  ->  rows 0:64 ctx,
             rows 64:128 softmax denominator (ones block), replicated
        rb = DVE reciprocal(den); ctxT = cps * rb  (2 strided mults)
      fc: F = ctxT.T @ W_fcT + residual
      LN1 -> x1 (token-major f32), transpose -> x1T bf16
      H1T = relu(W_1T.T @ x1T + b1)   [DVE max, feature-major]
      F2 = H1T.T @ W_2T (+b2) + x1; LN2 -> x2
  - out = x2[64:576]
"""

import numpy as np
import ml_dtypes

B, S, F_DIM, H, NH, HD, W_ONE, L = 2, 2048, 438, 768, 12, 64, 32, 2
NCORES = 8
CHUNK = 512          # output tokens per core
HALO = 64            # per side
T_LOC = CHUNK + 2 * HALO   # 640 local tokens
NT = T_LOC // 128          # 5 query tiles
KPAD = T_LOC + 64          # 704 padded key width
SPAN = 192                 # keys per query tile (128 + 2*32)
FK = 512                   # padded embedding contraction (438 -> 512)
# Large enough that masked positions contribute ~e^-50 ~ 2e-22 (negligible),
# small enough that a fully-masked (pad) query row keeps a nonzero softmax
# denominator -> no inf/NaN anywhere (pad rows are discarded on output).
MASK_NEG = -50.0

bf16 = ml_dtypes.bfloat16


def _np(x):
    return np.asarray(x)


def host_prep(inputs):
    """Split full inputs into shared weight arrays + per-core arrays."""
    src_seq = _np(inputs["src_seq"]).astype(np.float32)
    src_pos = _np(inputs["src_pos"]).astype(np.int32)
    pos_table = _np(inputs["pos_table"]).astype(np.float32)

    shared = {}
    qscale = float(HD) ** -0.5

    W_emb = _np(inputs["W_emb"]).astype(np.float32)        # [H, F]
    WembT = np.zeros((FK, H), np.float32)
    WembT[:F_DIM] = W_emb.T
    shared["wembT"] = WembT.astype(bf16)

    for l in range(L):
        Wq = _np(inputs["Wq"])[l].astype(np.float32)
        Wk = _np(inputs["Wk"])[l].astype(np.float32)
        Wv = _np(inputs["Wv"])[l].astype(np.float32)
        Wfc = _np(inputs["Wfc"])[l].astype(np.float32)
        W1 = _np(inputs["W1"])[l].astype(np.float32)
        W2 = _np(inputs["W2"])[l].astype(np.float32)
        shared[f"wqT{l}"] = (Wq.T * qscale).astype(bf16)   # [H_in, H_out]
        shared[f"wkT{l}"] = Wk.T.astype(bf16)
        shared[f"wvT{l}"] = Wv.T.astype(bf16)
        shared[f"wfcT{l}"] = Wfc.T.astype(bf16)
        shared[f"w1T{l}"] = W1.T.astype(bf16)
        shared[f"w2T{l}"] = W2.T.astype(bf16)
        shared[f"bq{l}"] = (_np(inputs["bq"])[l].astype(np.float32) * qscale)
        shared[f"bk{l}"] = _np(inputs["bk"])[l].astype(np.float32)
        shared[f"bv{l}"] = _np(inputs["bv"])[l].astype(np.float32)
        shared[f"bfc{l}"] = _np(inputs["bfc"])[l].astype(np.float32)
        shared[f"b1{l}"] = _np(inputs["b1"])[l].astype(np.float32)
        shared[f"b2{l}"] = _np(inputs["b2"])[l].astype(np.float32)
        shared[f"ln1g{l}"] = _np(inputs["ln1_g"])[l].astype(np.float32)
        shared[f"ln1b{l}"] = _np(inputs["ln1_b"])[l].astype(np.float32)
        shared[f"ln2g{l}"] = _np(inputs["ln2_g"])[l].astype(np.float32)
        shared[f"ln2b{l}"] = _np(inputs["ln2_b"])[l].astype(np.float32)

    b_emb = _np(inputs["b_emb"]).astype(np.float32)

    per_core = []
    for c in range(NCORES):
        b, q = divmod(c, NCORES // B)
        gstart = q * CHUNK - HALO
        lo, hi = max(gstart, 0), min(gstart + T_LOC, S)

        src_halo = np.zeros((T_LOC, F_DIM), np.float32)
        src_halo[lo - gstart: hi - gstart] = src_seq[b, lo:hi]
        srcT = np.zeros((FK, T_LOC), np.float32)
        srcT[:F_DIM] = src_halo.T

        pos_emb = np.zeros((T_LOC, H), np.float32)
        pos_emb[lo - gstart: hi - gstart] = pos_table[src_pos[b, lo:hi]]
        pos_emb += b_emb[None, :]

        # n-major masks per query tile: mask[n, q] for span keys
        # local key = 128t - 32 + n, query local = 128t + q.
        # mask4 layout [128, NT, 4, 256]: per head-slot j (4 identical copies
        # for the 4-head attention unit): cols 0:128 = hi block (keys
        # 128t-32..128t+96), cols 128:256 = lo block (keys 128t+96..128t+160
        # on rows 0:64; rows 64:128 permanently masked).
        mask4 = np.full((128, NT, 4, 256), MASK_NEG, np.float32)
        for t in range(NT):
            n = np.arange(SPAN)[:, None]
            qq = np.arange(128)[None, :]
            kl = 128 * t - 32 + n
            kg = gstart + kl
            band = np.abs(kl - (128 * t + qq)) <= W_ONE
            valid = band & (kl >= 0) & (kl < T_LOC) & (kg >= 0) & (kg < S)
            m = np.where(valid, 0.0, MASK_NEG).astype(np.float32)
            for j in range(4):
                mask4[:, t, j, 0:128] = m[:128]
                mask4[0:64, t, j, 128:256] = m[128:]

        per_core.append({
            "srcT": srcT.astype(bf16),
            "pos_emb": pos_emb,
            "mask4": np.ascontiguousarray(mask4.reshape(128, NT, 1024)).astype(bf16),
        })

    # constants
    shared["ident"] = np.eye(128, dtype=np.float32)
    shared["identb"] = np.eye(128, dtype=np.float32).astype(bf16)

    flags = {}
    for l in range(L):
        for nm in ("bq", "bk", "bv", "bfc", "b1", "b2"):
            flags[f"{nm}{l}"] = not np.allclose(shared[f"{nm}{l}"], 0.0)
        for nm in ("ln1", "ln2"):
            flags[f"{nm}{l}"] = not (
                np.allclose(shared[f"{nm}g{l}"], 1.0)
                and np.allclose(shared[f"{nm}b{l}"], 0.0)
            )
    return shared, per_core, flags


def assemble(core_outs):
    out = np.zeros((B, S, H), np.float32)
    for c in range(NCORES):
        b, q = divmod(c, NCORES // B)
        out[b, q * CHUNK:(q + 1) * CHUNK] = core_outs[c]
    return out


# ---------------------------------------------------------------------------
# Bass program
# ---------------------------------------------------------------------------

def _legalize_waits(nc):
    """This container's walrus codegen accepts only ONE sync-wait per compute
    instruction ("Too many sync wait commands"). Tile's scheduler emits
    multi-wait instructions, so split: keep the last wait on the instruction
    and carry earlier ones on same-engine NoOps inserted right before it."""
    import concourse.mybir as mybir

    for fn in nc.m.functions:
        for blk in fn.blocks:
            out = []
            changed = False
            for inst in blk.instructions:
                si = getattr(inst, "sync_info", None)
                waits = list(si.on_wait) if si is not None and si.on_wait else []
                if len(waits) > 1 and not isinstance(
                        inst, mybir.InstEventSemaphore):
                    for j, w in enumerate(waits[:-1]):
                        # NoOp lowers through the v3 codegen only; Activation
                        # and Pool go through v2 (no InstISA nop) -> use a
                        # 1-wait Drain there instead.
                        if inst.engine in (mybir.EngineType.Activation,
                                           mybir.EngineType.Pool):
                            nop = mybir.InstDrain(
                                name=f"{inst.name}-w{j}", ins=[], outs=[])
                        else:
                            nop = mybir.InstNoOp(
                                name=f"{inst.name}-w{j}", ins=[], outs=[])
                        nop.engine = inst.engine
                        nop.sync_info = mybir.SyncInfo(on_wait=[w], on_update=[])
                        out.append(nop)
                    inst.sync_info = mybir.SyncInfo(
                        on_wait=[waits[-1]], on_update=list(si.on_update or []))
                    changed = True
                out.append(inst)
            if changed:
                blk.instructions = out


def build_program(flags):
    import concourse.bass as bass
    import concourse.mybir as mybir
    import concourse.tile as tile

    f32 = mybir.dt.float32
    bf = mybir.dt.bfloat16
    AF = mybir.ActivationFunctionType
    ALU = mybir.AluOpType

    nc = bass.Bass()
    FT = H // 128          # 6 feature tiles
    KTE = FK // 128        # 4 embedding contraction tiles

    # ---- DRAM tensors ----
    D = {}
    names = []

    def din(name, shape, dt):
        D[name] = nc.dram_tensor(name, shape, dt, kind="ExternalInput")
        names.append(name)

    din("srcT", [FK, T_LOC], bf)
    din("pos_emb", [T_LOC, H], f32)
    din("mask4", [128, NT, 1024], bf)
    din("ident", [128, 128], f32)
    din("identb", [128, 128], bf)
    din("wembT", [FK, H], bf)
    for l in range(L):
        for nm in ("wqT", "wkT", "wvT", "wfcT", "w1T", "w2T"):
            din(f"{nm}{l}", [H, H], bf)
        for nm in ("bq", "bk", "bv", "bfc", "b1", "b2"):
            if flags[f"{nm}{l}"]:
                din(f"{nm}{l}", [H], f32)
        for nm in ("ln1", "ln2"):
            if flags[f"{nm}{l}"]:
                din(f"{nm}g{l}", [H], f32)
                din(f"{nm}b{l}", [H], f32)
    out_d = nc.dram_tensor("out", [CHUNK, H], f32, kind="ExternalOutput")

    def bcast_ap(dram, n):
        return bass.AP(tensor=dram.tensor, offset=dram.offset, ap=[[0, 128], [1, n]])

    with tile.TileContext(nc) as tc:
        import contextlib
        with contextlib.ExitStack() as ctx:
            consts = ctx.enter_context(tc.tile_pool(name="consts", bufs=1))
            acts = ctx.enter_context(tc.tile_pool(name="acts", bufs=1))
            work = ctx.enter_context(tc.tile_pool(name="work", bufs=2))
            ppool = ctx.enter_context(tc.tile_pool(name="pp", bufs=3))
            rpool = ctx.enter_context(tc.tile_pool(name="rp", bufs=2))
            spool = ctx.enter_context(tc.tile_pool(name="sp", bufs=4))
            # PSUM budget (8 banks x 2KB): psg 2x[128,512] = 2 banks,
            # stp 2x[128,1024] = 4 banks, cps 2x[128,512] = 2 banks.
            psg = ctx.enter_context(tc.tile_pool(name="psg", bufs=2, space="PSUM"))
            pstp = ctx.enter_context(tc.tile_pool(name="pstp", bufs=2, space="PSUM"))
            psc = ctx.enter_context(tc.tile_pool(name="psc", bufs=2, space="PSUM"))

            # ---- constants / inputs to SBUF ----
            ident_sb = consts.tile([128, 128], f32)
            nc.sync.dma_start(out=ident_sb, in_=D["ident"][:, :])
            identb_sb = consts.tile([128, 128], bf)
            nc.sync.dma_start(out=identb_sb, in_=D["identb"][:, :])
            eps_sb = consts.tile([128, 1], f32)
            nc.vector.memset(eps_sb, 1e-5)

            mask4_sb = consts.tile([128, NT, 1024], bf)
            nc.sync.dma_start(out=mask4_sb, in_=D["mask4"][:, :, :])

            srcT_sb = consts.tile([128, KTE, T_LOC], bf)
            for kt in range(KTE):
                nc.sync.dma_start(out=srcT_sb[:, kt, :],
                                  in_=D["srcT"][kt * 128:(kt + 1) * 128, :])
            pos_sb = consts.tile([128, NT, H], f32)
            for t in range(NT):
                nc.sync.dma_start(out=pos_sb[:, t, :],
                                  in_=D["pos_emb"][t * 128:(t + 1) * 128, :])

            # weights stream through a rotating pool: each is used once.
            # bufs=4 lets loads be issued a section ahead of their GEMM.
            wpool = ctx.enter_context(tc.tile_pool(name="wpool", bufs=4))

            def load_w(name, kt=FT):
                wt = wpool.tile([128, kt, H], bf, name=f"{name}_sb", tag="wt")
                for k in range(kt):
                    nc.sync.dma_start(out=wt[:, k, :],
                                      in_=D[name][k * 128:(k + 1) * 128, :])
                return wt

            BIAS = {}
            for l in range(L):
                for nm in ("bq", "bk", "b1"):  # per-partition, feature-major
                    if flags[f"{nm}{l}"]:
                        BIAS[f"{nm}{l}"] = consts.tile([128, FT], f32, name=f"{nm}{l}_sb")
                        nc.sync.dma_start(
                            out=BIAS[f"{nm}{l}"],
                            in_=D[f"{nm}{l}"].rearrange("(kt p) -> p kt", p=128))
                for nm in ("bv", "bfc", "b2"):  # broadcast, token-major
                    if flags[f"{nm}{l}"]:
                        BIAS[f"{nm}{l}"] = consts.tile([128, H], f32, name=f"{nm}{l}_sb")
                        nc.sync.dma_start(
                            out=BIAS[f"{nm}{l}"], in_=bcast_ap(D[f"{nm}{l}"], H))
                for nm in ("ln1", "ln2"):
                    if flags[f"{nm}{l}"]:
                        for gb in ("g", "b"):
                            BIAS[f"{nm}{gb}{l}"] = consts.tile([128, H], f32, name=f"{nm}{gb}{l}_sb")
                            nc.sync.dma_start(
                                out=BIAS[f"{nm}{gb}{l}"],
                                in_=bcast_ap(D[f"{nm}{gb}{l}"], H))

            # ---- persistent activations ----
            x_tok = acts.tile([128, NT, H], f32)          # token-major f32
            qT = acts.tile([128, FT, T_LOC], bf)
            kTp = acts.tile([128, FT, KPAD], bf)
            # V_aug: 6 shifted 128-token windows x 12 heads x [64 V | 64 ones]
            # (the ones block makes the ctx matmul also emit the softmax
            # denominator replicated across 64 partitions, for free in N)
            V_aug = acts.tile([128, FT, NH, 128], bf)
            for i in range(FT):
                nc.vector.memset(V_aug[:, i, :, 64:128], 1.0)
            ctxT = acts.tile([128, FT, T_LOC], bf)
            H1T = acts.tile([128, FT, T_LOC], bf)

            # xT: feature-major bf16 with 32-col zero pad on each side (cols
            # [32, 672) hold tokens [0, 640)); a fresh generation per
            # transpose-set so the pool tracks lifetimes.
            def new_xT(name):
                t_ = acts.tile([128, FT, H], bf, name=name, tag="xTslot")
                nc.vector.memset(t_[:, :, 0:32], 0.0)
                nc.vector.memset(t_[:, :, 32 + T_LOC:H], 0.0)
                return t_

            # ---- embedding ----
            wembT_sb = load_w("wembT", kt=KTE)
            W = {"wqT0": load_w("wqT0"), "wkT0": load_w("wkT0")}
            for t in range(NT):
                for c0 in (0, 384):
                    ps = psg.tile([128, 512], f32, name="psgemm", tag="gemm")[:, 0:384]
                    for kt in range(KTE):
                        nc.tensor.matmul(
                            ps, srcT_sb[:, kt, t * 128:(t + 1) * 128],
                            wembT_sb[:, kt, c0:c0 + 384],
                            start=(kt == 0), stop=(kt == KTE - 1))
                    nc.vector.tensor_add(
                        x_tok[:, t, c0:c0 + 384], ps, pos_sb[:, t, c0:c0 + 384])

            def transpose_set(dst, t, gscale=None, gbias=None):
                """PE-transpose x_tok tile t into dst[:, :, 128t:+128] (bf16)."""
                for g in range(2):
                    n_g = 4 if g == 0 else 2
                    trp = psg.tile([128, 512], f32, name="pstr", tag="gemm")
                    for j in range(n_g):
                        ft = 4 * g + j
                        nc.tensor.transpose(
                            trp[:, j * 128:(j + 1) * 128],
                            x_tok[:, t, ft * 128:(ft + 1) * 128], ident_sb)
                    src = trp[:, 0:n_g * 128].rearrange("p (a b) -> p a b", b=128)
                    nc.vector.tensor_copy(
                        dst[:, 4 * g:4 * g + n_g, 32 + t * 128:32 + (t + 1) * 128],
                        src)

            xT = new_xT("x0T")
            for t in range(NT):
                transpose_set(xT, t)

            # ---- layers ----
            for l in range(L):
                # kTp pad memsets
                nc.vector.memset(kTp[:, :, 0:32], 0.0)
                nc.vector.memset(kTp[:, :, 32 + T_LOC:KPAD], 0.0)

                # q/k GEMMs (feature-major outputs)
                wq, wk = W.pop(f"wqT{l}"), W.pop(f"wkT{l}")
                wv = load_w(f"wvT{l}")
                for ft in range(FT):
                    for c0 in (0, 320):
                        psq = psg.tile([128, 512], f32, name="psgemm", tag="gemm")[:, 0:384]
                        for kt in range(FT):
                            nc.tensor.matmul(
                                psq[:, 0:320], wq[:, kt, ft * 128:(ft + 1) * 128],
                                xT[:, kt, 32 + c0:32 + c0 + 320],
                                start=(kt == 0), stop=(kt == FT - 1))
                        if flags[f"bq{l}"]:
                            nc.vector.tensor_scalar_add(
                                qT[:, ft, c0:c0 + 320], psq[:, 0:320],
                                BIAS[f"bq{l}"][:, ft:ft + 1])
                        else:
                            nc.vector.tensor_copy(
                                qT[:, ft, c0:c0 + 320], psq[:, 0:320])
                        psk = psg.tile([128, 512], f32, name="psgemm", tag="gemm")[:, 0:384]
                        for kt in range(FT):
                            nc.tensor.matmul(
                                psk[:, 0:320], wk[:, kt, ft * 128:(ft + 1) * 128],
                                xT[:, kt, 32 + c0:32 + c0 + 320],
                                start=(kt == 0), stop=(kt == FT - 1))
                        if flags[f"bk{l}"]:
                            nc.vector.tensor_scalar_add(
                                kTp[:, ft, 32 + c0:32 + c0 + 320], psk[:, 0:320],
                                BIAS[f"bk{l}"][:, ft:ft + 1])
                        else:
                            nc.vector.tensor_copy(
                                kTp[:, ft, 32 + c0:32 + c0 + 320], psk[:, 0:320])

                # V GEMM: shifted token windows (xT is padded, so window i is
                # simply padded cols [128i, 128i+128) -> uniform M=128),
                # scattered into V_aug's per-head 128-col slots (64 V values;
                # cols 64:128 stay the ones block set up at init)
                wfc = load_w(f"wfcT{l}")
                for i in range(FT):
                    for c0 in (0, 384):
                        psv = psg.tile([128, 512], f32, name="psgemm", tag="gemm")[:, 0:384]
                        for kt in range(FT):
                            nc.tensor.matmul(
                                psv, xT[:, kt, 128 * i:128 * i + 128],
                                wv[:, kt, c0:c0 + 384],
                                start=(kt == 0), stop=(kt == FT - 1))
                        h0 = c0 // 64
                        dst = V_aug[:, i, h0:h0 + 6, 0:64]
                        src = psv.rearrange("p (h d) -> p h d", d=64)
                        if flags[f"bv{l}"]:
                            nc.vector.tensor_add(
                                dst, src,
                                BIAS[f"bv{l}"][:, c0:c0 + 384].rearrange(
                                    "p (h d) -> p h d", d=64))
                        else:
                            nc.vector.tensor_copy(dst, src)

                # attention: units of (query tile t, group g of 4 heads).
                # Scores live n-major in a [128, 4*256] psum tile; the band
                # mask is preloaded into psum by a bf16 identity matmul
                # (start=True) and the QK^T matmuls accumulate on top; one
                # exp covers the whole unit; ctx matmuls against V_aug give
                # ctx rows 0:64 and the replicated denominator rows 64:128.
                # slot order within a unit: slots {0,1} are the lower-half
                # heads (partitions 0:64) of feature tiles 2g, 2g+1; slots
                # {2,3} the upper halves -> the two normalize multiplies get
                # contiguous slot ranges
                HSLOT = (0, 2, 1, 3)
                w1 = load_w(f"w1T{l}")
                w2 = load_w(f"w2T{l}")
                for t in range(NT):
                    for g in range(3):
                        stp = pstp.tile([128, 1024], f32, tag="st")
                        for c0 in (0, 512):
                            nc.tensor.matmul(
                                stp[:, c0:c0 + 512], identb_sb,
                                mask4_sb[:, t, c0:c0 + 512],
                                start=True, stop=False, skip_group_check=True)
                        for j in range(4):
                            h = 4 * g + HSLOT[j]
                            hp, po = h // 2, 64 * (h % 2)
                            nc.tensor.matmul(
                                stp[:, 256 * j:256 * j + 128],
                                kTp[po:po + 64, hp, 128 * t:128 * t + 128],
                                qT[po:po + 64, hp, 128 * t:128 * t + 128],
                                start=False, stop=True, skip_group_check=True)
                            nc.tensor.matmul(
                                stp[0:64, 256 * j + 128:256 * j + 256],
                                kTp[po:po + 64, hp, 128 * t + 128:128 * t + 192],
                                qT[po:po + 64, hp, 128 * t:128 * t + 128],
                                start=False, stop=True, skip_group_check=True)
                        P = ppool.tile([128, 1024], bf, tag="p")
                        nc.scalar.activation(P, stp, AF.Exp)
                        cps = psc.tile([128, 4, 128], f32, tag="ctx")
                        for j in range(4):
                            h = 4 * g + HSLOT[j]
                            nc.tensor.matmul(
                                cps[:, j, :], V_aug[0:128, t, h, :],
                                P[:, 256 * j:256 * j + 128],
                                start=True, stop=False)
                            nc.tensor.matmul(
                                cps[:, j, :], V_aug[0:64, t + 1, h, :],
                                P[0:64, 256 * j + 128:256 * j + 256],
                                start=False, stop=True)
                        rb = rpool.tile([64, 4, 128], f32, tag="rb")
                        nc.vector.reciprocal(rb, cps[64:128, :, :])
                        for a in range(2):
                            nc.vector.tensor_tensor(
                                out=ctxT[64 * a:64 * a + 64, 2 * g:2 * g + 2,
                                         128 * t:128 * t + 128],
                                in0=cps[0:64, 2 * a:2 * a + 2, :],
                                in1=rb[:, 2 * a:2 * a + 2, :],
                                op=ALU.mult)

                # fc + residual + LN1 (+ transposes into a fresh xT gen)
                xT = new_xT(f"x1T{l}")
                for t in range(NT):
                    F = work.tile([128, H], f32, tag="F")
                    for c0 in (0, 384):
                        ps = psg.tile([128, 512], f32, name="psgemm", tag="gemm")[:, 0:384]
                        for kt in range(FT):
                            nc.tensor.matmul(
                                ps, ctxT[:, kt, 128 * t:128 * t + 128],
                                wfc[:, kt, c0:c0 + 384],
                                start=(kt == 0), stop=(kt == FT - 1))
                        nc.vector.tensor_add(
                            F[:, c0:c0 + 384], ps, x_tok[:, t, c0:c0 + 384])
                        if flags[f"bfc{l}"]:
                            nc.vector.tensor_add(
                                F[:, c0:c0 + 384], F[:, c0:c0 + 384],
                                BIAS[f"bfc{l}"][:, c0:c0 + 384])
                    _layernorm(nc, tc, spool, F, x_tok, t, eps_sb,
                               BIAS.get(f"ln1g{l}"), BIAS.get(f"ln1b{l}"),
                               f32, AF, ALU)
                    transpose_set(xT, t)  # x1T reuses the xT slot

                # FFN
                for ft in range(FT):
                    for c0 in (0, 320):
                        ps = psg.tile([128, 512], f32, name="psgemm", tag="gemm")[:, 0:384]
                        for kt in range(FT):
                            nc.tensor.matmul(
                                ps[:, 0:320], w1[:, kt, ft * 128:(ft + 1) * 128],
                                xT[:, kt, 32 + c0:32 + c0 + 320],
                                start=(kt == 0), stop=(kt == FT - 1))
                        if flags[f"b1{l}"]:
                            nc.vector.tensor_scalar(
                                out=H1T[:, ft, c0:c0 + 320], in0=ps[:, 0:320],
                                scalar1=BIAS[f"b1{l}"][:, ft:ft + 1],
                                scalar2=0.0, op0=ALU.add, op1=ALU.max)
                        else:
                            nc.vector.tensor_scalar_max(
                                H1T[:, ft, c0:c0 + 320], ps[:, 0:320], 0.0)
                if l < L - 1:
                    W[f"wqT{l+1}"] = load_w(f"wqT{l+1}")
                    W[f"wkT{l+1}"] = load_w(f"wkT{l+1}")
                    xT = new_xT(f"x2T{l}")
                for t in range(NT):
                    F2 = work.tile([128, H], f32, tag="F")
                    for c0 in (0, 384):
                        ps = psg.tile([128, 512], f32, name="psgemm", tag="gemm")[:, 0:384]
                        for kt in range(FT):
                            nc.tensor.matmul(
                                ps, H1T[:, kt, 128 * t:128 * t + 128],
                                w2[:, kt, c0:c0 + 384],
                                start=(kt == 0), stop=(kt == FT - 1))
                        nc.vector.tensor_add(
                            F2[:, c0:c0 + 384], ps, x_tok[:, t, c0:c0 + 384])
                        if flags[f"b2{l}"]:
                            nc.vector.tensor_add(
                                F2[:, c0:c0 + 384], F2[:, c0:c0 + 384],
                                BIAS[f"b2{l}"][:, c0:c0 + 384])
                    _layernorm(nc, tc, spool, F2, x_tok, t, eps_sb,
                               BIAS.get(f"ln2g{l}"), BIAS.get(f"ln2b{l}"),
                               f32, AF, ALU)
                    if l < L - 1:
                        transpose_set(xT, t)
                    else:
                        lo = max(128 * t, HALO) - 128 * t
                        hi = min(128 * t + 128, HALO + CHUNK) - 128 * t
                        nc.sync.dma_start(
                            out=out_d[128 * t + lo - HALO:128 * t + hi - HALO, :],
                            in_=x_tok[lo:hi, t, :])

    _legalize_waits(nc)
    return nc, names


def _layernorm(nc, tc, spool, F, x_tok, t, eps_sb, g_bc, b_bc, f32, AF, ALU):
    stats = spool.tile([128, 3, 6], f32, tag="stats")
    for sg in range(3):
        nc.vector.bn_stats(stats[:, sg, :], F[:, sg * 256:(sg + 1) * 256])
    mv = spool.tile([128, 2], f32, tag="mv")
    nc.vector.bn_aggr(mv, stats)
    sd = spool.tile([128, 1], f32, tag="sd")
    nc.scalar.activation(sd, mv[:, 1:2], AF.Sqrt, bias=eps_sb[:, 0:1])
    rstd = spool.tile([128, 1], f32, tag="rstd")
    nc.vector.reciprocal(rstd, sd)
    nc.vector.tensor_scalar(
        out=x_tok[:, t, :], in0=F, scalar1=mv[:, 0:1], scalar2=rstd,
        op0=ALU.subtract, op1=ALU.mult)
    if g_bc is not None:
        nc.vector.tensor_tensor(
            out=x_tok[:, t, :], in0=x_tok[:, t, :], in1=g_bc, op=ALU.mult)
        nc.vector.tensor_tensor(
            out=x_tok[:, t, :], in0=x_tok[:, t, :], in1=b_bc, op=ALU.add)


def run_on_device(shared, per_core, flags, trace=False):
    from concourse.bass_utils import run_bass_kernel_spmd

    nc, names = build_program(flags)
    in_maps = []
    for c in range(NCORES):
        m = {}
        for n in names:
            src = per_core[c] if n in per_core[c] else shared
            m[n] = np.ascontiguousarray(src[n])
        in_maps.append(m)
    res = run_bass_kernel_spmd(nc, in_maps, core_ids=list(range(NCORES)),
                               trace=trace)
    return [r["out"] for r in res.results], res


def kernel(**inputs):
    shared, per_core, flags = host_prep(inputs)
    core_outs, _ = run_on_device(shared, per_core, flags)
    return assemble(core_outs)


